# revision 17
# baseline (speedup 1.0000x reference)
"""Trainium2 Bass kernel for the AdreQwen2 MoE-LoRA SwiGLU MLP.

Problem (hardcoded): B=4, S=2048, H=2048, I=5504, E=8 experts, top-2
per-batch binary gating, rank-16 LoRA adapters on gate/up/down, scale 2.0.

Distribution: token-parallel across 8 NeuronCores (1024 tokens each; each
core's tokens belong to exactly one batch, so its 2 active experts are
fixed). The host pre-selects the top-2 experts per batch and folds the
LoRA adapters into the dense weights exactly (binary gates make this pure
linear algebra): W_eff = W + 2.0 * (A_e0|A_e1 @ B_e0|B_e1)^T. The device
kernel is then a pure dense SwiGLU MLP in bf16 (same 1 cycle/row PE rate
as f32r, but half the DMA bytes and 2x-fast FWL weight loads). No
collectives: outputs are disjoint token slices, concatenated on the host.

Device kernel (per core), fully fused — the intermediate h stays in SBUF:
  phase 1: h[i, t] = silu(Wg_eff x)[i, t] * (Wu_eff x)[i, t]
           43 I-tiles x 2 token chunks, contraction over H (16 K-tiles),
           f32 PSUM accumulate; h written to SBUF as bf16 (86KB/partition).
  phase 2: outT[o, t] = (Wd_eff h)[o, t]
           16 H-tiles x 2 token chunks, one 43-long accumulation chain
           per output tile straight out of SBUF-resident h. Wd is
           streamed (prefetched during phase 1), read exactly once.
"""

import sys
import types

import numpy as np

# ---- problem constants (must match setup_inputs) ----
B, S, H, I, E, R = 4, 2048, 2048, 5504, 8, 16
TOP_K = 2
LORA_SCALE = 32.0 / 16.0

P = 128
KH = H // P          # 16 K-tiles over H
KI = I // P          # 43 K-tiles / M-tiles over I
MH = H // P          # 16 M-tiles over H (phase 2 output)
N_CORES = 8
T = B * S            # 8192 tokens
T_CORE = T // N_CORES  # 1024 tokens per core
TCH = 512            # token chunk (matmul moving dim)
NCHUNK = T_CORE // TCH  # 2

_CACHE: dict = {}


def install_ntff_hook():
    """The antenv stub in this image lacks axon_hooks; reconstruct it so
    run_bass_kernel_spmd(trace=True) can capture NTFF profiles."""
    if "antenv.axon_hooks" in sys.modules:
        return
    try:
        mod = types.ModuleType("antenv.axon_hooks")
        mod._hook = None
        mod.set_axon_ntff_profile_hook = lambda h: setattr(mod, "_hook", h)
        mod.get_axon_ntff_profile_hook = lambda: mod._hook
        sys.modules["antenv.axon_hooks"] = mod
        from trn_agent_boot.trn_boot import _ntff_profile_via_ctypes

        mod.set_axon_ntff_profile_hook(
            _ntff_profile_via_ctypes("/opt/axon/libaxon_pjrt.so")
        )
    except Exception:
        sys.modules.pop("antenv.axon_hooks", None)


def _build_nc():
    import concourse.bacc as bacc
    import concourse.mybir as mybir
    import concourse.tile as tile
    from concourse.bass import ts

    f32 = mybir.dt.float32
    bf16 = mybir.dt.bfloat16
    silu_fn = mybir.ActivationFunctionType.Silu

    nc = bacc.Bacc()

    x_t = nc.declare_dram_parameter(
        "x_t", [P, NCHUNK, KH * TCH], bf16, isOutput=False
    )
    wg_t = nc.declare_dram_parameter("wg_t", [KI, P, KH * P], bf16, isOutput=False)
    wu_t = nc.declare_dram_parameter("wu_t", [KI, P, KH * P], bf16, isOutput=False)
    wd_t = nc.declare_dram_parameter("wd_t", [MH, P, KI * P], bf16, isOutput=False)
    outT = nc.declare_dram_parameter("outT", [H, T_CORE], f32, isOutput=True)

    with (
        tile.TileContext(nc) as tc,
        tc.tile_pool(name="xp", bufs=1) as xp,
        tc.tile_pool(name="hp", bufs=1) as hp,
        tc.tile_pool(name="cst", bufs=1) as cst,
        tc.tile_pool(name="work", bufs=3) as work,
        tc.tile_pool(name="wgp", bufs=3) as wgp,
        tc.tile_pool(name="wup", bufs=3) as wup,
        tc.tile_pool(name="wdp", bufs=3) as wdp,
        tc.tile_pool(name="outp", bufs=4) as outp,
        tc.tile_pool(name="psg", bufs=3, space="PSUM") as psg,
        tc.tile_pool(name="psup", bufs=3, space="PSUM") as psup,
        tc.tile_pool(name="pso", bufs=2, space="PSUM") as pso,
    ):
        # ---- PE warmup: dummy matmuls on a zeroed tile while the first
        # real DMAs are in flight, so the HAM clock-gate reaches 8/8
        # (2.4 GHz) before the first real matmul instead of ~3.4us after.
        warm_sb = cst.tile([P, TCH], bf16, tag="const", name="warm_sb")
        nc.vector.memset(warm_sb[:], 0)
        warm_ps = psg.tile([P, TCH], f32, tag="g", name="warm_ps")
        for w in range(18):
            nc.tensor.matmul(
                warm_ps[:], warm_sb[:, :P], warm_sb[:], start=True, stop=True
            )

        # ---- head DMAs. Only the sync (SP) and scalar (Activation) queues
        # have hardware descriptor generation; gpsimd DGE is software and
        # far too slow for the critical path. Interleave wg0 quarters with
        # chunk-0 x pairs on sync; wu0/x1/wg1/wu1 ride the scalar queue.
        # mi=0 gate weights as four SEPARATE tiles so the first LDWEIGHTS
        # only waits on quarter 0, not the whole wg0 transfer
        Q4 = 4 * P          # one wg0 quarter, flattened
        X4 = 4 * TCH        # one x quarter, flattened
        wg0q = [
            cst.tile([P, Q4], bf16, tag=f"wg0q{q}", name=f"wg0q_{q}")
            for q in range(4)
        ]
        wu0h = [
            cst.tile([P, KH * P // 2], bf16, tag=f"wu0h{q}", name=f"wu0h_{q}")
            for q in range(2)
        ]
        # chunk-0 x in 4-ko quarters (flat per-partition contiguous blocks
        # -> large DMA descriptors; progressive arrival)
        X_GROUPS = [(0, 2), (2, 4), (4, 8), (8, 12), (12, 16)]
        x0g = [
            xp.tile([P, (ke - ks) * TCH], bf16, tag=f"x0g{q}", name=f"x0g_{q}")
            for q, (ks, ke) in enumerate(X_GROUPS)
        ]
        x1 = xp.tile([P, KH * TCH], bf16, tag="x1", name="x1_sb")
        wg1 = wgp.tile([P, KH * P], bf16, tag="wg", name="wg_1")
        wu1 = wup.tile([P, KH * P], bf16, tag="wu", name="wu_1")

        # sync queue: wg0 quarters interleaved with the first x quarters,
        # then the second half of x1
        def xslice(q):
            ks, ke = X_GROUPS[q]
            return x_t[:, 0, ks * TCH : ke * TCH]

        nc.sync.dma_start(wg0q[0][:], wg_t[0][:, :Q4])
        nc.sync.dma_start(x0g[0][:], xslice(0))
        nc.sync.dma_start(x0g[1][:], xslice(1))
        nc.sync.dma_start(wg0q[1][:], wg_t[0][:, Q4 : 2 * Q4])
        nc.sync.dma_start(x0g[2][:], xslice(2))
        nc.sync.dma_start(wg0q[2][:], wg_t[0][:, 2 * Q4 : 3 * Q4])
        nc.sync.dma_start(wg0q[3][:], wg_t[0][:, 3 * Q4 :])
        nc.sync.dma_start(x1[:, KH * TCH // 2 :], x_t[:, 1, KH * TCH // 2 :])
        # scalar queue: wu0 halves, x0 tail, mi=1 weights, first half of x1
        nc.scalar.dma_start(wu0h[0][:], wu_t[0][:, : KH * P // 2])
        nc.scalar.dma_start(x0g[3][:], xslice(3))
        nc.scalar.dma_start(wu0h[1][:], wu_t[0][:, KH * P // 2 :])
        nc.scalar.dma_start(x0g[4][:], xslice(4))
        nc.scalar.dma_start(x1[:, : KH * TCH // 2], x_t[:, 1, : KH * TCH // 2])
        # mi=1 weights ride the (slow but otherwise idle) gpsimd SWDGE ring:
        # they are not needed until ~1 full chain-pass in, and this frees
        # scalar-ring bandwidth for the critical x pieces
        nc.gpsimd.dma_start(wg1[:], wg_t[1])
        nc.gpsimd.dma_start(wu1[:], wu_t[1])

        def xt(n, ko):
            if n == 0:
                for q, (ks, ke) in enumerate(X_GROUPS):
                    if ks <= ko < ke:
                        return x0g[q][:, (ko - ks) * TCH : (ko - ks + 1) * TCH]
            return x1[:, ko * TCH : (ko + 1) * TCH]

        def wgt(mi, wg_sb, ko):
            if mi == 0:
                return wg0q[ko // 4][:, (ko % 4) * P : (ko % 4 + 1) * P]
            return wg_sb[:, ko * P : (ko + 1) * P]

        def wut(mi, wu_sb, ko):
            if mi == 0:
                return wu0h[ko // 8][:, (ko % 8) * P : (ko % 8 + 1) * P]
            return wu_sb[:, ko * P : (ko + 1) * P]

        # ---------------- phase 1: h = silu(gate) * up (h stays in SBUF) ----
        wd_tiles = {}
        h_tiles = {}

        def gate_up(mi, n, wg_sb, wu_sb, interleave):
            pg = psg.tile([P, TCH], f32, tag="g", name=f"pg_{mi}_{n}")
            pup = psup.tile([P, TCH], f32, tag="up", name=f"pup_{mi}_{n}")
            if interleave:
                # gate/up interleaved at x-piece granularity so the head
                # chains track the progressive x0 arrival with 2x work per
                # arrived piece
                for ks, ke in X_GROUPS:
                    for ko in range(ks, ke):
                        nc.tensor.matmul(
                            pg[:], wgt(mi, wg_sb, ko), xt(n, ko),
                            start=(ko == 0), stop=(ko == KH - 1),
                        )
                    for ko in range(ks, ke):
                        nc.tensor.matmul(
                            pup[:], wut(mi, wu_sb, ko), xt(n, ko),
                            start=(ko == 0), stop=(ko == KH - 1),
                        )
            else:
                for ko in range(KH):
                    nc.tensor.matmul(
                        pg[:], wgt(mi, wg_sb, ko), xt(n, ko),
                        start=(ko == 0), stop=(ko == KH - 1),
                    )
                for ko in range(KH):
                    nc.tensor.matmul(
                        pup[:], wut(mi, wu_sb, ko), xt(n, ko),
                        start=(ko == 0), stop=(ko == KH - 1),
                    )
            sil = work.tile([P, TCH], f32, tag="sil", name=f"sil_{mi}_{n}")
            nc.scalar.activation(sil[:], pg[:], silu_fn)
            ht = hp.tile([P, TCH], bf16, tag=f"h{mi}_{n}", name=f"ht_{mi}_{n}")
            nc.vector.tensor_mul(out=ht[:], in0=sil[:], in1=pup[:])
            h_tiles[(mi, n)] = ht

        # head: chunk 0 of mi=0,1 while x0 trickles in; chunk 1 afterwards
        # (x1 lands during the chunk-0 passes)
        gate_up(0, 0, None, None, True)
        gate_up(1, 0, wg1, wu1, True)
        gate_up(0, 1, None, None, False)
        gate_up(1, 1, wg1, wu1, False)

        for mi in range(2, KI):
            wg_sb = wgp.tile([P, KH * P], bf16, tag="wg", name=f"wg_{mi}")
            nc.sync.dma_start(wg_sb[:], wg_t[mi])
            wu_sb = wup.tile([P, KH * P], bf16, tag="wu", name=f"wu_{mi}")
            nc.sync.dma_start(wu_sb[:], wu_t[mi])
            if mi in (4, 8, 12):
                # prefetch the first three Wd tiles mid-phase-1 (free
                # buffers, idle DMA bandwidth, away from the head burst)
                mh = (mi - 4) // 4
                wd_sb = wdp.tile([P, KI * P], bf16, tag="wd", name=f"wd_{mh}")
                nc.sync.dma_start(wd_sb[:], wd_t[mh])
                wd_tiles[mh] = wd_sb
            for n in range(NCHUNK):
                gate_up(mi, n, wg_sb, wu_sb, False)

        # ---------------- phase 2: outT = Wd h (h read from SBUF) ----------
        # One 43-long f32-PSUM accumulation chain per (mh, chunk). The very
        # last chain is split into two 256-column halves so the final
        # PSUM->SBUF copy + output DMA hide behind the second half's chain.
        for mh in range(MH):
            if mh in wd_tiles:
                wd_sb = wd_tiles[mh]
            else:
                wd_sb = wdp.tile([P, KI * P], bf16, tag="wd", name=f"wd_{mh}")
                nc.sync.dma_start(wd_sb[:], wd_t[mh])
            for n in range(NCHUNK):
                last = mh == MH - 1 and n == NCHUNK - 1
                subs = ((0, 256), (256, 384), (384, 512)) if last else ((0, TCH),)
                for lo, hi in subs:
                    po = pso.tile([P, hi - lo], f32, tag="o", name=f"po_{mh}_{n}_{lo}")
                    for ki in range(KI):
                        nc.tensor.matmul(
                            po[:],
                            wd_sb[:, ki * P : (ki + 1) * P],
                            h_tiles[(ki, n)][:, lo:hi],
                            start=(ki == 0),
                            stop=(ki == KI - 1),
                        )
                    ob = outp.tile([P, hi - lo], f32, tag="ob", name=f"ob_{mh}_{n}_{lo}")
                    nc.vector.tensor_copy(out=ob[:], in_=po[:])
                    nc.scalar.dma_start(
                        outT[ts(mh, P), n * TCH + lo : n * TCH + hi], ob[:]
                    )

    nc.finalize()
    return nc


def _get_nc():
    if "nc" not in _CACHE:
        _CACHE["nc"] = _build_nc()
    return _CACHE["nc"]


def _tile_kxm(w, n_m, n_k):
    """(M, K) row-major -> (n_m, P, n_k, P) with [mi, p, ko, m] = w[128mi+m, 128ko+p]."""
    return np.ascontiguousarray(w.reshape(n_m, P, n_k, P).transpose(0, 3, 2, 1))


def _prep_inputs(x, gate_values, Wg, Ag, Bg, Wu, Au, Bu, Wd, Ad, Bd):
    """Host-side expert selection, LoRA folding, sharding, and layout prep."""
    import ml_dtypes

    f32 = np.float32
    bf16 = ml_dtypes.bfloat16
    c = np.ascontiguousarray

    xf = np.asarray(x, f32).reshape(T, H)
    gv = np.asarray(gate_values, f32)
    idx = np.argsort(-gv, axis=1)[:, :TOP_K]  # (B, 2) top-2 experts per batch

    Wg_, Wu_, Wd_ = np.asarray(Wg, f32), np.asarray(Wu, f32), np.asarray(Wd, f32)
    Ag_, Bg_ = np.asarray(Ag, f32), np.asarray(Bg, f32)
    Au_, Bu_ = np.asarray(Au, f32), np.asarray(Bu, f32)
    Ad_, Bd_ = np.asarray(Ad, f32), np.asarray(Bd, f32)

    per_batch = []
    for b in range(B):
        es = [int(idx[b, 0]), int(idx[b, 1])]
        # exact LoRA fold: binary top-2 gates => W_eff = W + s * (A_cat @ B_cat)^T
        ag = np.concatenate([Ag_[e] for e in es], axis=1)  # (H, 2R)
        bg = np.concatenate([Bg_[e] for e in es], axis=0)  # (2R, I)
        au = np.concatenate([Au_[e] for e in es], axis=1)
        bu = np.concatenate([Bu_[e] for e in es], axis=0)
        ad = np.concatenate([Ad_[e] for e in es], axis=1)  # (I, 2R)
        bd = np.concatenate([Bd_[e] for e in es], axis=0)  # (2R, H)
        wg_eff = (Wg_ + LORA_SCALE * (ag @ bg).T).astype(bf16)  # (I, H)
        wu_eff = (Wu_ + LORA_SCALE * (au @ bu).T).astype(bf16)  # (I, H)
        wd_eff = (Wd_ + LORA_SCALE * (ad @ bd).T).astype(bf16)  # (H, I)
        per_batch.append(
            (
                _tile_kxm(wg_eff, KI, KH),
                _tile_kxm(wu_eff, KI, KH),
                _tile_kxm(wd_eff, MH, KI),
            )
        )

    in_maps = []
    for core in range(N_CORES):
        b = core * T_CORE // S  # batch this core's tokens belong to
        xc = xf[core * T_CORE : (core + 1) * T_CORE].astype(bf16)   # (1024, H)
        # [p, n, ko, t] = xc[n*TCH + t, ko*P + p] -> per-partition contiguous
        # 16KB chunk blocks (big DMA descriptors)
        x_tl = c(xc.reshape(NCHUNK, TCH, KH, P).transpose(3, 0, 2, 1))
        wg_tb, wu_tb, wd_tb = per_batch[b]
        in_maps.append({"x_t": x_tl, "wg_t": wg_tb, "wu_t": wu_tb, "wd_t": wd_tb})
    return in_maps


def _run(inputs, trace=False):
    from concourse.bass_utils import run_bass_kernel_spmd

    if trace:
        install_ntff_hook()
    nc = _get_nc()
    in_maps = _prep_inputs(**inputs)
    res = None
    last_err = None
    for attempt in range(3):  # transient NRT/axon execution errors are retriable
        try:
            res = run_bass_kernel_spmd(
                nc, in_maps, core_ids=list(range(N_CORES)), trace=trace
            )
            break
        except Exception as e:
            last_err = e
    if res is None:
        raise last_err
    outs = [res.results[c]["outT"] for c in range(N_CORES)]  # (H, 1024) each
    full = np.concatenate([o.T for o in outs], axis=0)       # (T, H)
    return full.reshape(B, S, H).astype(np.float32), res


def kernel(**inputs):
    out, _ = _run(inputs, trace=False)
    return out


# revision 18
# speedup vs baseline: 1.0022x; 1.0022x over previous
"""Trainium2 Bass kernel for the AdreQwen2 MoE-LoRA SwiGLU MLP.

Problem (hardcoded): B=4, S=2048, H=2048, I=5504, E=8 experts, top-2
per-batch binary gating, rank-16 LoRA adapters on gate/up/down, scale 2.0.

Distribution: token-parallel across 8 NeuronCores (1024 tokens each; each
core's tokens belong to exactly one batch, so its 2 active experts are
fixed). The host pre-selects the top-2 experts per batch and folds the
LoRA adapters into the dense weights exactly (binary gates make this pure
linear algebra): W_eff = W + 2.0 * (A_e0|A_e1 @ B_e0|B_e1)^T. The device
kernel is then a pure dense SwiGLU MLP in bf16 (same 1 cycle/row PE rate
as f32r, but half the DMA bytes and 2x-fast FWL weight loads). No
collectives: outputs are disjoint token slices, concatenated on the host.

Device kernel (per core), fully fused — the intermediate h stays in SBUF:
  phase 1: h[i, t] = silu(Wg_eff x)[i, t] * (Wu_eff x)[i, t]
           43 I-tiles x 2 token chunks, contraction over H (16 K-tiles),
           f32 PSUM accumulate; h written to SBUF as bf16 (86KB/partition).
  phase 2: outT[o, t] = (Wd_eff h)[o, t]
           16 H-tiles x 2 token chunks, one 43-long accumulation chain
           per output tile straight out of SBUF-resident h. Wd is
           streamed (prefetched during phase 1), read exactly once.
"""

import sys
import types

import numpy as np

# ---- problem constants (must match setup_inputs) ----
B, S, H, I, E, R = 4, 2048, 2048, 5504, 8, 16
TOP_K = 2
LORA_SCALE = 32.0 / 16.0

P = 128
KH = H // P          # 16 K-tiles over H
KI = I // P          # 43 K-tiles / M-tiles over I
MH = H // P          # 16 M-tiles over H (phase 2 output)
N_CORES = 8
T = B * S            # 8192 tokens
T_CORE = T // N_CORES  # 1024 tokens per core
TCH = 512            # token chunk (matmul moving dim)
NCHUNK = T_CORE // TCH  # 2

_CACHE: dict = {}


def install_ntff_hook():
    """The antenv stub in this image lacks axon_hooks; reconstruct it so
    run_bass_kernel_spmd(trace=True) can capture NTFF profiles."""
    if "antenv.axon_hooks" in sys.modules:
        return
    try:
        mod = types.ModuleType("antenv.axon_hooks")
        mod._hook = None
        mod.set_axon_ntff_profile_hook = lambda h: setattr(mod, "_hook", h)
        mod.get_axon_ntff_profile_hook = lambda: mod._hook
        sys.modules["antenv.axon_hooks"] = mod
        from trn_agent_boot.trn_boot import _ntff_profile_via_ctypes

        mod.set_axon_ntff_profile_hook(
            _ntff_profile_via_ctypes("/opt/axon/libaxon_pjrt.so")
        )
    except Exception:
        sys.modules.pop("antenv.axon_hooks", None)


def _build_nc():
    import concourse.bacc as bacc
    import concourse.mybir as mybir
    import concourse.tile as tile
    from concourse.bass import ts

    f32 = mybir.dt.float32
    bf16 = mybir.dt.bfloat16
    silu_fn = mybir.ActivationFunctionType.Silu

    nc = bacc.Bacc()

    x_t = nc.declare_dram_parameter(
        "x_t", [P, NCHUNK, KH * TCH], bf16, isOutput=False
    )
    wg_t = nc.declare_dram_parameter("wg_t", [KI, P, KH * P], bf16, isOutput=False)
    wu_t = nc.declare_dram_parameter("wu_t", [KI, P, KH * P], bf16, isOutput=False)
    wd_t = nc.declare_dram_parameter("wd_t", [MH, P, KI * P], bf16, isOutput=False)
    outT = nc.declare_dram_parameter("outT", [H, T_CORE], f32, isOutput=True)

    with (
        tile.TileContext(nc) as tc,
        tc.tile_pool(name="xp", bufs=1) as xp,
        tc.tile_pool(name="hp", bufs=1) as hp,
        tc.tile_pool(name="cst", bufs=1) as cst,
        tc.tile_pool(name="work", bufs=3) as work,
        tc.tile_pool(name="wgp", bufs=3) as wgp,
        tc.tile_pool(name="wup", bufs=3) as wup,
        tc.tile_pool(name="wdp", bufs=3) as wdp,
        tc.tile_pool(name="outp", bufs=4) as outp,
        tc.tile_pool(name="psg", bufs=3, space="PSUM") as psg,
        tc.tile_pool(name="psup", bufs=3, space="PSUM") as psup,
        tc.tile_pool(name="pso", bufs=2, space="PSUM") as pso,
    ):
        # ---- PE warmup: dummy matmuls on a zeroed tile while the first
        # real DMAs are in flight, so the HAM clock-gate reaches 8/8
        # (2.4 GHz) before the first real matmul instead of ~3.4us after.
        warm_sb = cst.tile([P, TCH], bf16, tag="const", name="warm_sb")
        nc.vector.memset(warm_sb[:], 0)
        warm_ps = psg.tile([P, TCH], f32, tag="g", name="warm_ps")
        for w in range(18):
            nc.tensor.matmul(
                warm_ps[:], warm_sb[:, :P], warm_sb[:], start=True, stop=True
            )

        # ---- head DMAs. Only the sync (SP) and scalar (Activation) queues
        # have hardware descriptor generation; gpsimd DGE is software and
        # far too slow for the critical path. Interleave wg0 quarters with
        # chunk-0 x pairs on sync; wu0/x1/wg1/wu1 ride the scalar queue.
        # mi=0 gate weights as four SEPARATE tiles so the first LDWEIGHTS
        # only waits on quarter 0, not the whole wg0 transfer
        Q4 = 4 * P          # one wg0 quarter, flattened
        X4 = 4 * TCH        # one x quarter, flattened
        wg0q = [
            cst.tile([P, Q4], bf16, tag=f"wg0q{q}", name=f"wg0q_{q}")
            for q in range(4)
        ]
        wu0h = [
            cst.tile([P, KH * P // 2], bf16, tag=f"wu0h{q}", name=f"wu0h_{q}")
            for q in range(2)
        ]
        # chunk-0 x in 4-ko quarters (flat per-partition contiguous blocks
        # -> large DMA descriptors; progressive arrival)
        X_GROUPS = [(0, 2), (2, 4), (4, 8), (8, 12), (12, 16)]
        x0g = [
            xp.tile([P, (ke - ks) * TCH], bf16, tag=f"x0g{q}", name=f"x0g_{q}")
            for q, (ks, ke) in enumerate(X_GROUPS)
        ]
        x1 = xp.tile([P, KH * TCH], bf16, tag="x1", name="x1_sb")
        wg1 = wgp.tile([P, KH * P], bf16, tag="wg", name="wg_1")
        wu1 = wup.tile([P, KH * P], bf16, tag="wu", name="wu_1")

        # sync queue: wg0 quarters interleaved with the first x quarters,
        # then the second half of x1
        def xslice(q):
            ks, ke = X_GROUPS[q]
            return x_t[:, 0, ks * TCH : ke * TCH]

        nc.sync.dma_start(wg0q[0][:], wg_t[0][:, :Q4])
        nc.sync.dma_start(x0g[0][:], xslice(0))
        nc.sync.dma_start(x0g[1][:], xslice(1))
        nc.sync.dma_start(wg0q[1][:], wg_t[0][:, Q4 : 2 * Q4])
        nc.sync.dma_start(x0g[2][:], xslice(2))
        nc.sync.dma_start(wg0q[2][:], wg_t[0][:, 2 * Q4 : 3 * Q4])
        nc.sync.dma_start(wg0q[3][:], wg_t[0][:, 3 * Q4 :])
        nc.sync.dma_start(x1[:, KH * TCH // 2 :], x_t[:, 1, KH * TCH // 2 :])
        # scalar queue: wu0 halves, x0 tail, mi=1 weights, first half of x1
        nc.scalar.dma_start(wu0h[0][:], wu_t[0][:, : KH * P // 2])
        nc.scalar.dma_start(x0g[3][:], xslice(3))
        nc.scalar.dma_start(wu0h[1][:], wu_t[0][:, KH * P // 2 :])
        nc.scalar.dma_start(x0g[4][:], xslice(4))
        nc.scalar.dma_start(wg1[:], wg_t[1])
        nc.scalar.dma_start(wu1[:], wu_t[1])
        nc.scalar.dma_start(x1[:, : KH * TCH // 2], x_t[:, 1, : KH * TCH // 2])

        def xt(n, ko):
            if n == 0:
                for q, (ks, ke) in enumerate(X_GROUPS):
                    if ks <= ko < ke:
                        return x0g[q][:, (ko - ks) * TCH : (ko - ks + 1) * TCH]
            return x1[:, ko * TCH : (ko + 1) * TCH]

        def wgt(mi, wg_sb, ko):
            if mi == 0:
                return wg0q[ko // 4][:, (ko % 4) * P : (ko % 4 + 1) * P]
            return wg_sb[:, ko * P : (ko + 1) * P]

        def wut(mi, wu_sb, ko):
            if mi == 0:
                return wu0h[ko // 8][:, (ko % 8) * P : (ko % 8 + 1) * P]
            return wu_sb[:, ko * P : (ko + 1) * P]

        # ---------------- phase 1: h = silu(gate) * up (h stays in SBUF) ----
        wd_tiles = {}
        h_tiles = {}

        def gate_up(mi, n, wg_sb, wu_sb, interleave):
            pg = psg.tile([P, TCH], f32, tag="g", name=f"pg_{mi}_{n}")
            pup = psup.tile([P, TCH], f32, tag="up", name=f"pup_{mi}_{n}")
            if interleave:
                # gate/up interleaved at x-piece granularity so the head
                # chains track the progressive x0 arrival with 2x work per
                # arrived piece
                for ks, ke in X_GROUPS:
                    for ko in range(ks, ke):
                        nc.tensor.matmul(
                            pg[:], wgt(mi, wg_sb, ko), xt(n, ko),
                            start=(ko == 0), stop=(ko == KH - 1),
                        )
                    for ko in range(ks, ke):
                        nc.tensor.matmul(
                            pup[:], wut(mi, wu_sb, ko), xt(n, ko),
                            start=(ko == 0), stop=(ko == KH - 1),
                        )
            else:
                for ko in range(KH):
                    nc.tensor.matmul(
                        pg[:], wgt(mi, wg_sb, ko), xt(n, ko),
                        start=(ko == 0), stop=(ko == KH - 1),
                    )
                for ko in range(KH):
                    nc.tensor.matmul(
                        pup[:], wut(mi, wu_sb, ko), xt(n, ko),
                        start=(ko == 0), stop=(ko == KH - 1),
                    )
            sil = work.tile([P, TCH], f32, tag="sil", name=f"sil_{mi}_{n}")
            nc.scalar.activation(sil[:], pg[:], silu_fn)
            ht = hp.tile([P, TCH], bf16, tag=f"h{mi}_{n}", name=f"ht_{mi}_{n}")
            nc.vector.tensor_mul(out=ht[:], in0=sil[:], in1=pup[:])
            h_tiles[(mi, n)] = ht

        # head: chunk 0 of mi=0,1 while x0 trickles in; chunk 1 afterwards
        # (x1 lands during the chunk-0 passes)
        gate_up(0, 0, None, None, True)
        gate_up(1, 0, wg1, wu1, True)
        gate_up(0, 1, None, None, False)
        gate_up(1, 1, wg1, wu1, False)

        for mi in range(2, KI):
            wg_sb = wgp.tile([P, KH * P], bf16, tag="wg", name=f"wg_{mi}")
            nc.sync.dma_start(wg_sb[:], wg_t[mi])
            wu_sb = wup.tile([P, KH * P], bf16, tag="wu", name=f"wu_{mi}")
            nc.sync.dma_start(wu_sb[:], wu_t[mi])
            if mi in (4, 8, 12):
                # prefetch the first three Wd tiles mid-phase-1 (free
                # buffers, idle DMA bandwidth, away from the head burst)
                mh = (mi - 4) // 4
                wd_sb = wdp.tile([P, KI * P], bf16, tag="wd", name=f"wd_{mh}")
                nc.sync.dma_start(wd_sb[:], wd_t[mh])
                wd_tiles[mh] = wd_sb
            for n in range(NCHUNK):
                gate_up(mi, n, wg_sb, wu_sb, False)

        # ---------------- phase 2: outT = Wd h (h read from SBUF) ----------
        # One 43-long f32-PSUM accumulation chain per (mh, chunk). The very
        # last chain is split into two 256-column halves so the final
        # PSUM->SBUF copy + output DMA hide behind the second half's chain.
        for mh in range(MH):
            if mh in wd_tiles:
                wd_sb = wd_tiles[mh]
            else:
                wd_sb = wdp.tile([P, KI * P], bf16, tag="wd", name=f"wd_{mh}")
                nc.sync.dma_start(wd_sb[:], wd_t[mh])
            for n in range(NCHUNK):
                last = mh == MH - 1 and n == NCHUNK - 1
                subs = ((0, 256), (256, 384), (384, 512)) if last else ((0, TCH),)
                for lo, hi in subs:
                    po = pso.tile([P, hi - lo], f32, tag="o", name=f"po_{mh}_{n}_{lo}")
                    for ki in range(KI):
                        nc.tensor.matmul(
                            po[:],
                            wd_sb[:, ki * P : (ki + 1) * P],
                            h_tiles[(ki, n)][:, lo:hi],
                            start=(ki == 0),
                            stop=(ki == KI - 1),
                        )
                    ob = outp.tile([P, hi - lo], f32, tag="ob", name=f"ob_{mh}_{n}_{lo}")
                    nc.vector.tensor_copy(out=ob[:], in_=po[:])
                    nc.scalar.dma_start(
                        outT[ts(mh, P), n * TCH + lo : n * TCH + hi], ob[:]
                    )

    nc.finalize()
    return nc


def _get_nc():
    if "nc" not in _CACHE:
        _CACHE["nc"] = _build_nc()
    return _CACHE["nc"]


def _tile_kxm(w, n_m, n_k):
    """(M, K) row-major -> (n_m, P, n_k, P) with [mi, p, ko, m] = w[128mi+m, 128ko+p]."""
    return np.ascontiguousarray(w.reshape(n_m, P, n_k, P).transpose(0, 3, 2, 1))


def _prep_inputs(x, gate_values, Wg, Ag, Bg, Wu, Au, Bu, Wd, Ad, Bd):
    """Host-side expert selection, LoRA folding, sharding, and layout prep."""
    import ml_dtypes

    f32 = np.float32
    bf16 = ml_dtypes.bfloat16
    c = np.ascontiguousarray

    xf = np.asarray(x, f32).reshape(T, H)
    gv = np.asarray(gate_values, f32)
    idx = np.argsort(-gv, axis=1)[:, :TOP_K]  # (B, 2) top-2 experts per batch

    Wg_, Wu_, Wd_ = np.asarray(Wg, f32), np.asarray(Wu, f32), np.asarray(Wd, f32)
    Ag_, Bg_ = np.asarray(Ag, f32), np.asarray(Bg, f32)
    Au_, Bu_ = np.asarray(Au, f32), np.asarray(Bu, f32)
    Ad_, Bd_ = np.asarray(Ad, f32), np.asarray(Bd, f32)

    per_batch = []
    for b in range(B):
        es = [int(idx[b, 0]), int(idx[b, 1])]
        # exact LoRA fold: binary top-2 gates => W_eff = W + s * (A_cat @ B_cat)^T
        ag = np.concatenate([Ag_[e] for e in es], axis=1)  # (H, 2R)
        bg = np.concatenate([Bg_[e] for e in es], axis=0)  # (2R, I)
        au = np.concatenate([Au_[e] for e in es], axis=1)
        bu = np.concatenate([Bu_[e] for e in es], axis=0)
        ad = np.concatenate([Ad_[e] for e in es], axis=1)  # (I, 2R)
        bd = np.concatenate([Bd_[e] for e in es], axis=0)  # (2R, H)
        wg_eff = (Wg_ + LORA_SCALE * (ag @ bg).T).astype(bf16)  # (I, H)
        wu_eff = (Wu_ + LORA_SCALE * (au @ bu).T).astype(bf16)  # (I, H)
        wd_eff = (Wd_ + LORA_SCALE * (ad @ bd).T).astype(bf16)  # (H, I)
        per_batch.append(
            (
                _tile_kxm(wg_eff, KI, KH),
                _tile_kxm(wu_eff, KI, KH),
                _tile_kxm(wd_eff, MH, KI),
            )
        )

    in_maps = []
    for core in range(N_CORES):
        b = core * T_CORE // S  # batch this core's tokens belong to
        xc = xf[core * T_CORE : (core + 1) * T_CORE].astype(bf16)   # (1024, H)
        # [p, n, ko, t] = xc[n*TCH + t, ko*P + p] -> per-partition contiguous
        # 16KB chunk blocks (big DMA descriptors)
        x_tl = c(xc.reshape(NCHUNK, TCH, KH, P).transpose(3, 0, 2, 1))
        wg_tb, wu_tb, wd_tb = per_batch[b]
        in_maps.append({"x_t": x_tl, "wg_t": wg_tb, "wu_t": wu_tb, "wd_t": wd_tb})
    return in_maps


def _run(inputs, trace=False):
    from concourse.bass_utils import run_bass_kernel_spmd

    if trace:
        install_ntff_hook()
    nc = _get_nc()
    in_maps = _prep_inputs(**inputs)
    res = None
    last_err = None
    for attempt in range(3):  # transient NRT/axon execution errors are retriable
        try:
            res = run_bass_kernel_spmd(
                nc, in_maps, core_ids=list(range(N_CORES)), trace=trace
            )
            break
        except Exception as e:
            last_err = e
    if res is None:
        raise last_err
    outs = [res.results[c]["outT"] for c in range(N_CORES)]  # (H, 1024) each
    full = np.concatenate([o.T for o in outs], axis=0)       # (T, H)
    return full.reshape(B, S, H).astype(np.float32), res


def kernel(**inputs):
    out, _ = _run(inputs, trace=False)
    return out
